# revision 10
# baseline (speedup 1.0000x reference)
"""Trainium2 Bass kernel for nn_HConstructor_for_visual.

Computes, for features [N=50000, D=256], T=3 extra views, E=512 edges:
  transformed views -> backbone MLP -> per-view argmax hyperedge assignment
  -> Hc scatter counts -> H softmax over nodes, hyperedge pooling, dots.

Sharding: node dimension N across 8 NeuronCores (6250 nodes/core, padded to
6272 = 49*128). Three device programs:

  K1: fp32r (tf32-like) chain feat->transform->L0->L1->logits, per-row top-8
      max + argmax via DVE max/max_index. Also exports F^T and raw af2 (view
      0) scratch in fp32r for K3. Host flags near-tie rows (top1-top2 gap
      < TAU) whose fp32r argmax may disagree with exact fp32.
  K2: exact-fp32 recompute of the full chain for up to 2048 flagged nodes
      (256/core) -> corrected argmax.
  K3: from final classes: Hc via iota-compare accumulation, mask=sign(Hc),
      exp(Hc); per-shard hyperedge pooling mask^T @ af2 and colsum(exp(Hc))
      partials -> single packed AllReduce -> H = exp(Hc)/S, G_v = W_v @ hf^T
      folding for dots = F @ G_v (exploits b_lin == 0), dots written directly
      row-major.

kernel(**inputs) takes full unsharded inputs, returns (H, hyperedge_features,
dots) matching reference.py.
"""
import numpy as np

import concourse.bass as bass
import concourse.mybir as mybir
import concourse.tile as tile
from concourse import bacc
from concourse.bass_utils import run_bass_kernel_spmd
from concourse.masks import make_identity

F32 = mybir.dt.float32
F32R = mybir.dt.float32r
F16 = mybir.dt.float16
U32 = mybir.dt.uint32
I32 = mybir.dt.int32
AF = mybir.ActivationFunctionType
ALU = mybir.AluOpType
AX = mybir.AxisListType

P = 128
D = 256
DK = D // P          # 2 feature-dim chunks
E = 512
EM = E // P          # 4 edge-dim chunks
T = 3
V = T + 1
N = 50000
NCORES = 8
NS = N // NCORES     # 6250 nodes per core
SUBS = (NS + P - 1) // P      # 49 node subtiles of 128
NSP = SUBS * P       # 6272 padded
NPAD = NSP - NS      # 22 zero-pad rows per core
SCALE = float(D) ** -0.5

TAU = 3e-3           # near-tie flag threshold on top1-top2 logit gap
RFIX_C = 256         # K2 capacity per core (2048 total flagged nodes)

_prog_cache: dict = {}
LAST_EXEC_NS: list = []   # per-launch HW exec time (ns) when tracing is on
LAST_RESULTS: list = []   # per-launch BassKernelResults (for trace analysis)


def _run(nc, in_maps, core_ids):
    res = run_bass_kernel_spmd(nc, in_maps, core_ids=core_ids)
    ns = res.exec_time_ns
    if ns is None and res.mean_exec_time_ns is not None:
        ns = int(res.mean_exec_time_ns)
    LAST_EXEC_NS.append(ns)
    LAST_RESULTS.append(res)
    return res


def _chunks(subs):
    """Groups of node-subtiles processed together (up to 4 subs = 512 cols)."""
    out = []
    s = 0
    while s < subs:
        g = min(4, subs - s)
        out.append((s, g))
        s += g
    return out


# --------------------------------------------------------------------------
# K1: fp32r chain + argmax/top2
# --------------------------------------------------------------------------
def _build_k1(subs=SUBS, ncores=NCORES):
    nsp = subs * P
    nc = bacc.Bacc("TRN2", target_bir_lowering=False, debug=False,
                   num_devices=ncores)
    feat = nc.dram_tensor("feat", [nsp, D], F32, kind="ExternalInput").ap()
    wlin = nc.dram_tensor("wlin", [T, D, D], F32, kind="ExternalInput").ap()
    w0 = nc.dram_tensor("w0", [D, D], F32, kind="ExternalInput").ap()
    w1 = nc.dram_tensor("w1", [D, D], F32, kind="ExternalInput").ap()
    we = nc.dram_tensor("we", [D, E], F32, kind="ExternalInput").ap()
    vals8 = nc.dram_tensor("vals8", [V, P, subs * 8], F32,
                           kind="ExternalOutput").ap()
    idx8 = nc.dram_tensor("idx8", [V, P, subs * 8], U32,
                          kind="ExternalOutput").ap()
    fts = nc.dram_tensor("fts", [D, nsp], F32R, kind="ExternalOutput").ap()
    af2s = nc.dram_tensor("af2s", [nsp, D], F32R, kind="ExternalOutput").ap()

    with tile.TileContext(nc) as tc:
        with tc.tile_pool(name="wp", bufs=1) as wp, \
             tc.tile_pool(name="io", bufs=2) as io, \
             tc.tile_pool(name="ac", bufs=2) as ac, \
             tc.tile_pool(name="st", bufs=1) as st, \
             tc.tile_pool(name="psA", bufs=2, space="PSUM") as psA, \
             tc.tile_pool(name="psB", bufs=2, space="PSUM") as psB:

            ident = wp.tile([P, P], F32, tag="id")
            make_identity(nc, ident[:])

            # ---- weights -> SBUF, cast to fp32r ----
            def load_r(name, src, width):
                # src [D, width]: k-chunk rows -> [P, DK*width] fp32r
                t32 = io.tile([P, DK * width], F32, tag="wtmp")
                for k in range(DK):
                    nc.sync.dma_start(t32[:, k * width:(k + 1) * width],
                                      src[k * P:(k + 1) * P, :])
                tr = wp.tile([P, DK * width], F32R, tag=name)
                nc.vector.tensor_copy(tr[:], t32[:])
                return tr

            # wl: cols (v, k) -> [P, D] block of W_lin[v][k*128:(k+1)*128, :]
            wl = wp.tile([P, T * DK * D], F32R, tag="wl")
            for v in range(T):
                for k in range(DK):
                    t32 = io.tile([P, D], F32, tag="wtmp")
                    nc.sync.dma_start(t32[:], wlin[v, k * P:(k + 1) * P, :])
                    nc.vector.tensor_copy(
                        wl[:, (v * DK + k) * D:(v * DK + k + 1) * D], t32[:])
            w0c = load_r("w0c", w0, D)
            w1c = load_r("w1c", w1, D)
            wec = load_r("wec", we, E)

            vstage = [st.tile([P, subs * 8], F32, tag=f"vs{v}",
                              name=f"vstage{v}") for v in range(V)]
            istage = [st.tile([P, subs * 8], U32, tag=f"is{v}",
                              name=f"istage{v}") for v in range(V)]

            for (s0, nsub) in _chunks(subs):
                cn = nsub * P
                off = s0 * P
                # load F rows [cn, D]
                fch = io.tile([P, 4 * D], F32, tag="fch")
                for i in range(nsub):
                    nc.sync.dma_start(
                        fch[:, i * D:(i + 1) * D],
                        feat[off + i * P: off + (i + 1) * P, :])
                # transpose -> F^T fp32r [DK][P, cn]; also relu copy
                ftr = []
                r0r = []
                for j in range(DK):
                    pt = psA.tile([P, 512], F32, tag="ftp")
                    for i in range(nsub):
                        nc.tensor.transpose(
                            pt[:, i * P:(i + 1) * P],
                            fch[:, i * D + j * P: i * D + (j + 1) * P],
                            ident[:])
                    f_r = ac.tile([P, 512], F32R, tag=f"ft{j}")
                    nc.any.tensor_copy(f_r[:, :cn], pt[:, :cn])
                    ftr.append(f_r)
                    r_r = ac.tile([P, 512], F32R, tag=f"r0{j}")
                    nc.any.tensor_relu(r_r[:, :cn], pt[:, :cn])
                    r0r.append(r_r)
                    nc.sync.dma_start(fts[j * P:(j + 1) * P, off:off + cn],
                                      f_r[:, :cn])

                # transforms (views 1..T): rv[v][m] = relu(F @ Wv + b)^T chunk
                rv = {}
                for v in range(T):
                    for m in range(DK):
                        pm = psA.tile([P, 512], F32, tag="mm")
                        for k in range(DK):
                            nc.tensor.matmul(
                                pm[:, :cn],
                                wl[:, (v * DK + k) * D + m * P:
                                   (v * DK + k) * D + (m + 1) * P],
                                ftr[k][:, :cn],
                                start=(k == 0), stop=(k == DK - 1))
                        t = ac.tile([P, 512], F32R, tag=f"rv{v}{m}")
                        nc.any.tensor_relu(t[:, :cn], pm[:, :cn])
                        rv[(v, m)] = t

                # L0: r1[v][m] = relu(rin @ W0)^T
                r1 = {}
                for v in range(V):
                    rin = r0r if v == 0 else [rv[(v - 1, m)] for m in range(DK)]
                    for m in range(DK):
                        pm = psA.tile([P, 512], F32, tag="mm")
                        for k in range(DK):
                            nc.tensor.matmul(
                                pm[:, :cn],
                                w0c[:, k * D + m * P: k * D + (m + 1) * P],
                                rin[k][:, :cn],
                                start=(k == 0), stop=(k == DK - 1))
                        t = ac.tile([P, 512], F32R, tag=f"r1{v}{m}")
                        nc.any.tensor_relu(t[:, :cn], pm[:, :cn])
                        r1[(v, m)] = t

                # L1: r2[v][m] = relu(r1 @ W1)^T ; also raw af2 view0 rows
                r2 = {}
                for v in range(V):
                    for m in range(DK):
                        pm = psA.tile([P, 512], F32, tag="mm")
                        for k in range(DK):
                            nc.tensor.matmul(
                                pm[:, :cn],
                                w1c[:, k * D + m * P: k * D + (m + 1) * P],
                                r1[(v, k)][:, :cn],
                                start=(k == 0), stop=(k == DK - 1))
                        t = ac.tile([P, 512], F32R, tag=f"r2{v}{m}")
                        nc.any.tensor_relu(t[:, :cn], pm[:, :cn])
                        r2[(v, m)] = t

                # af2 view0 raw, node-major (for K3 pooling): af2n[s] [P, D]
                for i in range(nsub):
                    pa = psB.tile([P, 512], F32, tag="afp")
                    for k in range(DK):
                        nc.tensor.matmul(
                            pa[:, :D],
                            r1[(0, k)][:, i * P:(i + 1) * P],
                            w1c[:, k * D:(k + 1) * D],
                            start=(k == 0), stop=(k == DK - 1))
                    aa = io.tile([P, D], F32R, tag="aa")
                    nc.any.tensor_copy(aa[:], pa[:, :D])
                    nc.sync.dma_start(af2s[off + i * P: off + (i + 1) * P, :],
                                      aa[:])

                # logits + argmax per view, node-subtile
                for v in range(V):
                    for i in range(nsub):
                        pl = psB.tile([P, 512], F32, tag="lg")
                        for k in range(DK):
                            nc.tensor.matmul(
                                pl[:],
                                r2[(v, k)][:, i * P:(i + 1) * P],
                                wec[:, k * E:(k + 1) * E],
                                start=(k == 0), stop=(k == DK - 1))
                        si = s0 + i
                        nc.vector.max(vstage[v][:, si * 8:(si + 1) * 8], pl[:])
                        nc.vector.max_index(
                            istage[v][:, si * 8:(si + 1) * 8],
                            vstage[v][:, si * 8:(si + 1) * 8], pl[:])

            for v in range(V):
                nc.sync.dma_start(vals8[v], vstage[v][:])
                nc.sync.dma_start(idx8[v], istage[v][:])

    nc.compile()
    return nc


# --------------------------------------------------------------------------
# K2: exact fp32 chain for flagged nodes
# --------------------------------------------------------------------------
def _build_k2(rows=RFIX_C, ncores=NCORES):
    nsub = rows // P     # 2
    nc = bacc.Bacc("TRN2", target_bir_lowering=False, debug=False,
                   num_devices=ncores)
    feat = nc.dram_tensor("feat", [rows, D], F32, kind="ExternalInput").ap()
    wlin = nc.dram_tensor("wlin", [T, D, D], F32, kind="ExternalInput").ap()
    w0 = nc.dram_tensor("w0", [D, D], F32, kind="ExternalInput").ap()
    w1 = nc.dram_tensor("w1", [D, D], F32, kind="ExternalInput").ap()
    we = nc.dram_tensor("we", [D, E], F32, kind="ExternalInput").ap()
    idxf = nc.dram_tensor("idxf", [P, V * nsub * 8], U32,
                          kind="ExternalOutput").ap()

    cn = rows
    with tile.TileContext(nc) as tc:
        with tc.tile_pool(name="wp", bufs=1) as wp, \
             tc.tile_pool(name="io", bufs=2) as io, \
             tc.tile_pool(name="ps", bufs=2, space="PSUM") as ps:
            ident = wp.tile([P, P], F32, tag="id")
            make_identity(nc, ident[:])

            wl = wp.tile([P, T * DK * D], F32, tag="wl")
            for v in range(T):
                for k in range(DK):
                    nc.sync.dma_start(
                        wl[:, (v * DK + k) * D:(v * DK + k + 1) * D],
                        wlin[v, k * P:(k + 1) * P, :])
            w0c = wp.tile([P, DK * D], F32, tag="w0c")
            w1c = wp.tile([P, DK * D], F32, tag="w1c")
            wec = wp.tile([P, DK * E], F32, tag="wec")
            for k in range(DK):
                nc.sync.dma_start(w0c[:, k * D:(k + 1) * D],
                                  w0[k * P:(k + 1) * P, :])
                nc.sync.dma_start(w1c[:, k * D:(k + 1) * D],
                                  w1[k * P:(k + 1) * P, :])
                nc.sync.dma_start(wec[:, k * E:(k + 1) * E],
                                  we[k * P:(k + 1) * P, :])

            ist = wp.tile([P, V * nsub * 8], U32, tag="ist")
            vst = wp.tile([P, V * nsub * 8], F32, tag="vst")

            fch = io.tile([P, nsub * D], F32, tag="fch")
            for i in range(nsub):
                nc.sync.dma_start(fch[:, i * D:(i + 1) * D],
                                  feat[i * P:(i + 1) * P, :])
            ftr, r0r = [], []
            for j in range(DK):
                pt = ps.tile([P, cn], F32, tag="ftp")
                for i in range(nsub):
                    nc.tensor.transpose(
                        pt[:, i * P:(i + 1) * P],
                        fch[:, i * D + j * P: i * D + (j + 1) * P], ident[:])
                f32t = io.tile([P, cn], F32, tag=f"ft{j}")
                nc.any.tensor_copy(f32t[:], pt[:])
                ftr.append(f32t)
                r_r = io.tile([P, cn], F32, tag=f"r0{j}")
                nc.any.tensor_relu(r_r[:], pt[:])
                r0r.append(r_r)

            rv = {}
            for v in range(T):
                for m in range(DK):
                    pm = ps.tile([P, cn], F32, tag="mm")
                    for k in range(DK):
                        nc.tensor.matmul(
                            pm[:],
                            wl[:, (v * DK + k) * D + m * P:
                               (v * DK + k) * D + (m + 1) * P],
                            ftr[k][:], start=(k == 0), stop=(k == DK - 1))
                    t = io.tile([P, cn], F32, tag=f"rv{v}{m}")
                    nc.any.tensor_relu(t[:], pm[:])
                    rv[(v, m)] = t
            r1 = {}
            for v in range(V):
                rin = r0r if v == 0 else [rv[(v - 1, m)] for m in range(DK)]
                for m in range(DK):
                    pm = ps.tile([P, cn], F32, tag="mm")
                    for k in range(DK):
                        nc.tensor.matmul(
                            pm[:], w0c[:, k * D + m * P: k * D + (m + 1) * P],
                            rin[k][:], start=(k == 0), stop=(k == DK - 1))
                    t = io.tile([P, cn], F32, tag=f"r1{v}{m}")
                    nc.any.tensor_relu(t[:], pm[:])
                    r1[(v, m)] = t
            r2 = {}
            for v in range(V):
                for m in range(DK):
                    pm = ps.tile([P, cn], F32, tag="mm")
                    for k in range(DK):
                        nc.tensor.matmul(
                            pm[:], w1c[:, k * D + m * P: k * D + (m + 1) * P],
                            r1[(v, k)][:], start=(k == 0), stop=(k == DK - 1))
                    t = io.tile([P, cn], F32, tag=f"r2{v}{m}")
                    nc.any.tensor_relu(t[:], pm[:])
                    r2[(v, m)] = t
            for v in range(V):
                for i in range(nsub):
                    pl = ps.tile([P, E], F32, tag="lg")
                    for k in range(DK):
                        nc.tensor.matmul(
                            pl[:], r2[(v, k)][:, i * P:(i + 1) * P],
                            wec[:, k * E:(k + 1) * E],
                            start=(k == 0), stop=(k == DK - 1))
                    col = (v * nsub + i) * 8
                    nc.vector.max(vst[:, col:col + 8], pl[:])
                    nc.vector.max_index(ist[:, col:col + 8],
                                        vst[:, col:col + 8], pl[:])
            nc.sync.dma_start(idxf, ist[:])
    nc.compile()
    return nc


# --------------------------------------------------------------------------
# K3: Hc/H, pooling + AllReduce, G fold, dots
# --------------------------------------------------------------------------
def _build_k3(subs=SUBS, ncores=NCORES, npad_total=NCORES * NPAD):
    nsp = subs * P
    nc = bacc.Bacc("TRN2", target_bir_lowering=False, debug=False,
                   num_devices=ncores)
    fts = nc.dram_tensor("fts", [D, nsp], F32R, kind="ExternalInput").ap()
    af2s = nc.dram_tensor("af2s", [nsp, D], F32R, kind="ExternalInput").ap()
    cls = nc.dram_tensor("cls", [P, V * subs], I32, kind="ExternalInput").ap()
    wlin = nc.dram_tensor("wlin", [T, D, D], F32, kind="ExternalInput").ap()
    H = nc.dram_tensor("H", [nsp, E], F32, kind="ExternalOutput").ap()
    dots = nc.dram_tensor("dots", [V, nsp, E], F32, kind="ExternalOutput").ap()
    hf = nc.dram_tensor("hf", [E, D], F32, kind="ExternalOutput").ap()

    with tile.TileContext(nc) as tc:
        with tc.tile_pool(name="wp", bufs=1) as wp, \
             tc.tile_pool(name="big", bufs=1) as big, \
             tc.tile_pool(name="io", bufs=3) as io, \
             tc.tile_pool(name="wk", bufs=2) as wk, \
             tc.tile_pool(name="ps", bufs=2, space="PSUM") as ps, \
             tc.tile_pool(name="pacc", bufs=1, space="PSUM") as pacc, \
             tc.tile_pool(name="dram", bufs=1, space="DRAM") as dram:

            ident = wp.tile([P, P], F32, tag="id")
            make_identity(nc, ident[:])
            ones32 = wp.tile([P, 1], F32, tag="ones32")
            nc.vector.memset(ones32[:], 1.0)
            iota_i = wp.tile([P, E], I32, tag="ioi")
            nc.gpsimd.iota(iota_i[:], pattern=[[1, E]], base=0,
                           channel_multiplier=0)
            iota_f = wp.tile([P, E], F32, tag="iof")
            nc.vector.tensor_copy(iota_f[:], iota_i[:])

            # W_lin^T chunks for G_v: wlt col (v,k,m) holds
            # transpose(W_v[m*128:(m+1)*128, k*128:(k+1)*128]) = [dout k, din m]
            wlt = wp.tile([P, T * DK * DK * P], F32R, tag="wlt")
            for v in range(T):
                for k in range(DK):
                    for m in range(DK):
                        t32 = io.tile([P, P], F32, tag="wtmp")
                        nc.sync.dma_start(
                            t32[:], wlin[v, m * P:(m + 1) * P,
                                         k * P:(k + 1) * P])
                        pt = ps.tile([P, 512], F32, tag="tp")
                        nc.tensor.transpose(pt[:, :P], t32[:], ident[:])
                        nc.any.tensor_copy(
                            wlt[:, ((v * DK + k) * DK + m) * P:
                                ((v * DK + k) * DK + m + 1) * P], pt[:, :P])

            # classes -> fp32
            cls_sb = wp.tile([P, V * subs], I32, tag="clsi")
            nc.sync.dma_start(cls_sb[:], cls)
            cls_f = wp.tile([P, V * subs], F32, tag="clsf")
            nc.vector.tensor_copy(cls_f[:], cls_sb[:])

            # persistent big tiles
            ftbig = big.tile([P, DK * nsp], F32R, tag="ftbig")
            for k in range(DK):
                nc.sync.dma_start(ftbig[:, k * nsp:(k + 1) * nsp],
                                  fts[k * P:(k + 1) * P, :])
            ubig = big.tile([P, subs * E], F16, tag="ubig")   # Hc as f16

            # persistent PSUM accumulators: pooling [e-chunk][P, D], S [1, E]
            hfp = [pacc.tile([P, D], F32, tag=f"hfp{m}", name=f"hfp{m}")
                   for m in range(EM)]
            sp = pacc.tile([1, E], F32, tag="sp")

            # ---- sweep 1: Hc, mask, exp, pooling + S partials ----
            for s in range(subs):
                an = io.tile([P, D], F32R, tag="an")
                nc.sync.dma_start(an[:], af2s[s * P:(s + 1) * P, :])
                hc = wk.tile([P, E], F32, tag="hc")
                nc.vector.tensor_scalar(hc[:], iota_f[:],
                                        cls_f[:, 0 * subs + s: 0 * subs + s + 1],
                                        None, op0=ALU.is_equal)
                for v in range(1, V - 1):
                    nc.vector.scalar_tensor_tensor(
                        hc[:], iota_f[:],
                        cls_f[:, v * subs + s: v * subs + s + 1], hc[:],
                        op0=ALU.is_equal, op1=ALU.add)
                # last eq writes f16 Hc into ubig
                uslice = ubig[:, s * E:(s + 1) * E]
                nc.vector.scalar_tensor_tensor(
                    uslice, iota_f[:],
                    cls_f[:, (V - 1) * subs + s: (V - 1) * subs + s + 1], hc[:],
                    op0=ALU.is_equal, op1=ALU.add)
                mask_r = wk.tile([P, E], F32R, tag="mk")
                nc.scalar.activation(mask_r[:], uslice, AF.Sign)
                exp32 = wk.tile([P, E], F32, tag="ex")
                nc.scalar.activation(exp32[:], uslice, AF.Exp)
                nc.tensor.matmul(sp[:], ones32[:], exp32[:],
                                 start=(s == 0), stop=(s == subs - 1))
                for m in range(EM):
                    nc.tensor.matmul(hfp[m][:],
                                     mask_r[:, m * P:(m + 1) * P], an[:],
                                     start=(s == 0), stop=(s == subs - 1))

            # ---- pack partials, AllReduce, unpack ----
            hfsb = wp.tile([P, EM * D], F32, tag="hfsb")
            for m in range(EM):
                nc.any.tensor_copy(hfsb[:, m * D:(m + 1) * D], hfp[m][:])
            srow = wp.tile([1, E], F32, tag="srow")
            nc.any.tensor_copy(srow[:], sp[:])

            cin = dram.tile([P, EM * D + 4], F32)
            cout = dram.tile([P, EM * D + 4], F32)
            nc.sync.dma_start(cin[:, :EM * D], hfsb[:])
            # S row [1,512] -> [128,4] region; row-major orders match
            nc.sync.dma_start(cin[:, EM * D:EM * D + 4], srow[:])
            nc.gpsimd.collective_compute(
                "AllReduce", ALU.add,
                replica_groups=[list(range(ncores))],
                ins=[cin.opt()], outs=[cout.opt()])
            hfall = wp.tile([P, EM * D], F32, tag="hfall")
            nc.sync.dma_start(hfall[:], cout[:, :EM * D])
            srow2 = wp.tile([1, E], F32, tag="srow2")
            nc.sync.dma_start(srow2[:], cout[:, EM * D:EM * D + 4])

            # hf output rows
            for m in range(EM):
                nc.sync.dma_start(hf[m * P:(m + 1) * P, :],
                                  hfall[:, m * D:(m + 1) * D])

            # S -> recip broadcast (subtract pad-row contribution)
            nc.vector.tensor_scalar(srow2[:], srow2[:], float(-npad_total),
                                    None, op0=ALU.add)
            rec = wp.tile([1, E], F32, tag="rec")
            nc.vector.reciprocal(rec[:], srow2[:])
            recb = wp.tile([P, E], F32, tag="recb")
            nc.gpsimd.partition_broadcast(recb[:], rec[:])

            # hf^T fp32r [DK][P, E]
            hftr = []
            for k in range(DK):
                pt = ps.tile([P, 512], F32, tag="tp")
                for m in range(EM):
                    nc.tensor.transpose(
                        pt[:, m * P:(m + 1) * P],
                        hfall[:, m * D + k * P: m * D + (k + 1) * P],
                        ident[:])
                t = wp.tile([P, E], F32R, tag=f"hft{k}")
                nc.any.tensor_copy(t[:], pt[:])
                hftr.append(t)

            # G_v = W_v @ hf^T, fp32r [T][DK][P, E]
            gr = {}
            for v in range(T):
                for m in range(DK):
                    pg = ps.tile([P, 512], F32, tag="tp")
                    for k in range(DK):
                        nc.tensor.matmul(
                            pg[:],
                            wlt[:, ((v * DK + k) * DK + m) * P:
                                ((v * DK + k) * DK + m + 1) * P],
                            hftr[k][:], start=(k == 0), stop=(k == DK - 1))
                    t = wp.tile([P, E], F32R, tag=f"g{v}{m}")
                    nc.any.tensor_copy(t[:], pg[:])
                    gr[(v, m)] = t

            # ---- sweep 2: dots + H ----
            for s in range(subs):
                for v in range(V):
                    pd = ps.tile([P, 512], F32, tag="tp")
                    for k in range(DK):
                        rhs = hftr[k] if v == 0 else gr[(v - 1, k)]
                        nc.tensor.matmul(
                            pd[:], ftbig[:, k * nsp + s * P: k * nsp + (s + 1) * P],
                            rhs[:], start=(k == 0), stop=(k == DK - 1))
                    db = io.tile([P, E], F32, tag="db")
                    nc.any.tensor_scalar(db[:], pd[:], SCALE, None,
                                         op0=ALU.mult)
                    nc.sync.dma_start(dots[v, s * P:(s + 1) * P, :], db[:])
                eh = wk.tile([P, E], F32, tag="eh")
                nc.scalar.activation(eh[:], ubig[:, s * E:(s + 1) * E], AF.Exp)
                hb = io.tile([P, E], F32, tag="hb")
                nc.vector.tensor_tensor(hb[:], eh[:], recb[:], op=ALU.mult)
                nc.sync.dma_start(H[s * P:(s + 1) * P, :], hb[:])

    nc.compile()
    return nc


# --------------------------------------------------------------------------
# host orchestration
# --------------------------------------------------------------------------
def _get_prog(name, builder):
    if name not in _prog_cache:
        _prog_cache[name] = builder()
    return _prog_cache[name]


def kernel(features, W_lin, b_lin, W0, b0, W1, b1, We, be):
    features = np.ascontiguousarray(np.asarray(features, np.float32))
    W_lin = np.ascontiguousarray(np.asarray(W_lin, np.float32))
    W0 = np.ascontiguousarray(np.asarray(W0, np.float32))
    W1 = np.ascontiguousarray(np.asarray(W1, np.float32))
    We = np.ascontiguousarray(np.asarray(We, np.float32))
    for b in (b_lin, b0, b1, be):
        assert not np.asarray(b).any(), "nonzero biases unsupported"
    assert features.shape == (N, D)

    cores = list(range(NCORES))

    # ---- K1 ----
    nc1 = _get_prog("k1", _build_k1)
    featp = np.zeros((NCORES, NSP, D), np.float32)
    featp[:, :NS] = features.reshape(NCORES, NS, D)
    in1 = [{"feat": featp[c], "wlin": W_lin, "w0": W0, "w1": W1, "we": We}
           for c in cores]
    LAST_EXEC_NS.clear()
    r1 = _run(nc1, in1, cores)

    cls = np.zeros((NCORES, V, NSP), np.int64)
    gap = np.zeros((NCORES, V, NSP), np.float32)
    for c in cores:
        v8 = r1.results[c]["vals8"].reshape(V, P, SUBS, 8)
        i8 = r1.results[c]["idx8"].reshape(V, P, SUBS, 8)
        cls[c] = np.transpose(i8[..., 0], (0, 2, 1)).reshape(V, NSP)
        gap[c] = np.transpose(v8[..., 0] - v8[..., 1], (0, 2, 1)).reshape(V, NSP)

    # ---- flag near-ties, K2 exact fixup ----
    flag = (gap[:, :, :NS] < TAU).any(axis=1)          # [NCORES, NS]
    flat_nodes = np.nonzero(flag.reshape(-1))[0]       # global node ids
    if len(flat_nodes) > NCORES * RFIX_C:
        # extremely unlikely; exact host recompute for all flagged
        cls_fix = _numpy_exact_classes(features[flat_nodes], W_lin, W0, W1, We)
        for j, n in enumerate(flat_nodes):
            c, ln = divmod(n, NS)
            cls[c, :, ln] = cls_fix[:, j]
    elif len(flat_nodes) > 0:
        nc2 = _get_prog("k2", _build_k2)
        nfix = len(flat_nodes)
        ffix = np.zeros((NCORES, RFIX_C, D), np.float32)
        fx = features[flat_nodes]
        for c in cores:
            part = fx[c * RFIX_C:(c + 1) * RFIX_C]
            ffix[c, :len(part)] = part
        in2 = [{"feat": ffix[c], "wlin": W_lin, "w0": W0, "w1": W1, "we": We}
               for c in cores]
        r2 = _run(nc2, in2, cores)
        nsub2 = RFIX_C // P
        for c in cores:
            i0 = c * RFIX_C
            cnt = min(max(nfix - i0, 0), RFIX_C)
            if cnt == 0:
                break
            arr = r2.results[c]["idxf"].reshape(P, V, nsub2, 8)[..., 0]
            # arr[p, v, s] -> row s*P + p
            arr = np.transpose(arr, (1, 2, 0)).reshape(V, RFIX_C)
            for j in range(cnt):
                n = flat_nodes[i0 + j]
                c2, ln = divmod(int(n), NS)
                cls[c2, :, ln] = arr[:, j]

    # ---- K3 ----
    nc3 = _get_prog("k3", _build_k3)
    cls_k3 = np.full((NCORES, P, V, SUBS), -1, np.int32)
    cls_valid = cls.copy()
    cls_valid[:, :, NS:] = -1                     # pad rows contribute nothing
    for c in cores:
        cls_k3[c] = np.transpose(
            cls_valid[c].reshape(V, SUBS, P), (2, 0, 1))
    in3 = [{"fts": r1.results[c]["fts"], "af2s": r1.results[c]["af2s"],
            "cls": cls_k3[c].reshape(P, V * SUBS), "wlin": W_lin}
           for c in cores]
    r3 = _run(nc3, in3, cores)

    H = np.empty((N, E), np.float32)
    dots = np.empty((V, N, E), np.float32)
    for c in cores:
        H[c * NS:(c + 1) * NS] = r3.results[c]["H"][:NS]
        dots[:, c * NS:(c + 1) * NS] = r3.results[c]["dots"][:, :NS]
    hyperedge_features = r3.results[0]["hf"]
    return H, hyperedge_features, dots.reshape(V * N, E)


def _numpy_exact_classes(f, W_lin, W0, W1, We):
    f = f.astype(np.float64)
    out = []
    for v in range(V):
        x = f if v == 0 else f @ W_lin[v - 1].astype(np.float64)
        x = np.maximum(x, 0) @ W0.astype(np.float64)
        x = np.maximum(x, 0) @ W1.astype(np.float64)
        lg = np.maximum(x, 0) @ We.astype(np.float64)
        out.append(lg.argmax(1))
    return np.stack(out)   # [V, rows]
